# revision 20
# baseline (speedup 1.0000x reference)
"""Per-sample Chamfer distance (B=16, NB=4096, 3D) on 8 TRN2 NeuronCores.

Data-parallel: 2 samples per core.  Per sample, the 4096x4096 squared
pairwise-distance matrix is produced on the TensorEngine as two bf16
matmuls per (128 x 512) tile (hi/lo-split operands, exact bf16 products
accumulated in f32 PSUM):

    d2[i,j] = a2[i] + b2[j] - 2*a[i].b[j]
            ~ MM1(K=7): -2ah.bh + 1*b2h + 1*b2l + a2h*1 + a2l*1
            + MM2(K=6): -2ah.bl + -2al.bh

ScalarE evacuates each PSUM tile to SBUF bf16 with Relu (the reference's
max(d2, 0) clamp).  VectorE then computes, per i-tile, the forward
row-min via a bf16 tensor_tensor min fold-chain (2x perf mode) and the
backward column-min by elementwise min-accumulation.  The backward
accumulator is finished with PE transposes + free-dim reduces, and both
directions get sqrt + mean on ScalarE.  min/sqrt commute, so reducing in
the d2 domain is exact up to bf16 rounding of the min itself.
"""

import os

import numpy as np
import ml_dtypes

B = 16
NB = 4096
D = 3
NCORES = 8
SPC = B // NCORES  # samples per core

BF16 = ml_dtypes.bfloat16

_CACHE = {}


def _build_nc():
    import concourse.bass as bass
    import concourse.bacc as bacc
    import concourse.mybir as mybir
    from concourse.tile import TileContext
    from contextlib import ExitStack

    dt = mybir.dt
    AF = mybir.ActivationFunctionType
    ALU = mybir.AluOpType
    AX = mybir.AxisListType

    NIT = NB // 128  # 32 i-tiles
    NJB = NB // 512  # 8 j-blocks

    nc = bacc.Bacc()

    amat = nc.declare_dram_parameter("amat", [SPC, 24, NB], dt.bfloat16, isOutput=False)
    bmat = nc.declare_dram_parameter("bmat", [SPC, 24, NB], dt.bfloat16, isOutput=False)
    ident = nc.declare_dram_parameter("ident", [128, 128], dt.bfloat16, isOutput=False)
    spat = nc.declare_dram_parameter("spat", [SPC, 128, NIT], dt.float32, isOutput=True)
    scal = nc.declare_dram_parameter("scal", [SPC, 1], dt.float32, isOutput=True)

    with TileContext(nc) as tc, ExitStack() as ctx:
        sb = ctx.enter_context(tc.tile_pool(name="sb", bufs=1))
        evp = ctx.enter_context(tc.tile_pool(name="ev", bufs=3))
        fp = ctx.enter_context(tc.tile_pool(name="fold", bufs=2))
        pp = ctx.enter_context(tc.tile_pool(name="ps", bufs=4, space="PSUM"))

        idt = sb.tile([128, 128], dt.bfloat16, tag="idt")
        nc.sync.dma_start(out=idt[:, :], in_=ident[:, :])
        ones_t = sb.tile([128, 1], dt.float32, tag="ones")
        nc.vector.memset(ones_t[:], 1.0)

        ta = []
        tb = []
        fwdmin = []
        acc = []
        for s in range(SPC):
            ta_s = sb.tile([128, NB], dt.bfloat16, tag=f"ta{s}")
            tb_s = sb.tile([128, NB], dt.bfloat16, tag=f"tb{s}")
            # replicate the 24 host rows at base partitions 0/32/64/96 (so
            # consecutive j-blocks run in four distinct PE row-groups,
            # concurrently).  Column-chunked DMAs: any 128-column weight
            # slice is covered by ONE dma (LDWEIGHTS caps sync-waits <4).
            for dram, tile in ((amat, ta_s), (bmat, tb_s)):
                for c in range(0, NB, 1024):
                    for o in (0, 32, 64, 96):
                        nc.sync.dma_start(out=tile[o:o + 24, c:c + 1024],
                                          in_=dram[s, :, c:c + 1024])
            ta.append(ta_s)
            tb.append(tb_s)
            fwdmin.append(sb.tile([128, NIT], dt.float32, tag=f"fw{s}", name=f"fw{s}"))
            acc.append(sb.tile([128, NB], dt.bfloat16, tag=f"acc{s}", name=f"acc{s}"))

        for s in range(SPC):
            ta_s, tb_s, fw_s, acc_s = ta[s], tb[s], fwdmin[s], acc[s]
            for it in range(NIT):
                isl = slice(it * 128, (it + 1) * 128)
                ev = evp.tile([128, NB], dt.bfloat16, tag="ev")
                for jb in range(NJB):
                    jsl = slice(jb * 512, (jb + 1) * 512)
                    # rotate PE row-groups 0-3 across j-blocks (concurrent MMs)
                    o = 32 * (jb % 4)
                    ps = pp.tile([128, 512], dt.float32, tag="d2", bufs=6)
                    nc.tensor.matmul(ps[:, :], lhsT=ta_s[o:o + 24, isl],
                                     rhs=tb_s[o:o + 24, jsl],
                                     start=True, stop=True,
                                     tile_position=(o, 0))
                    nc.scalar.activation(out=ev[:, jsl], in_=ps[:, :], func=AF.Relu)
                # backward: elementwise min-accumulate across i-tiles
                if it == 0:
                    nc.vector.tensor_copy(acc_s[:, :], ev[:, :])
                else:
                    nc.vector.tensor_tensor(out=acc_s[:, :], in0=ev[:, :],
                                            in1=acc_s[:, :], op=ALU.min)
                # forward: bf16 fold chain (2x mode) down to 256, then reduce
                f1 = fp.tile([128, 2048], dt.bfloat16, tag="f1")
                nc.vector.tensor_tensor(out=f1[:, :], in0=ev[:, 0:2048],
                                        in1=ev[:, 2048:4096], op=ALU.min)
                f2 = fp.tile([128, 1024], dt.bfloat16, tag="f2")
                nc.vector.tensor_tensor(out=f2[:, :], in0=f1[:, 0:1024],
                                        in1=f1[:, 1024:2048], op=ALU.min)
                f3 = fp.tile([128, 512], dt.bfloat16, tag="f3")
                nc.vector.tensor_tensor(out=f3[:, :], in0=f2[:, 0:512],
                                        in1=f2[:, 512:1024], op=ALU.min)
                f4 = fp.tile([128, 256], dt.bfloat16, tag="f4")
                nc.vector.tensor_tensor(out=f4[:, :], in0=f3[:, 0:256],
                                        in1=f3[:, 256:512], op=ALU.min)
                nc.vector.tensor_reduce(out=fw_s[:, it:it + 1], in_=f4[:, :],
                                        axis=AX.X, op=ALU.min)

        # backward finish: PE-transpose the accumulator, reduce along free dim
        bwdT = []
        for s in range(SPC):
            bw_s = sb.tile([128, NIT], dt.float32, tag=f"bw{s}")
            for g in range(4):
                tp = pp.tile([128, 1024], dt.bfloat16, tag="tp", bufs=1)
                for t8 in range(8):
                    t = g * 8 + t8
                    nc.tensor.transpose(tp[:, t8 * 128:(t8 + 1) * 128],
                                        acc[s][:, t * 128:(t + 1) * 128], idt[:, :])
                nc.vector.tensor_reduce(
                    out=bw_s[:, g * 8:(g + 1) * 8],
                    in_=tp[:, :].rearrange("p (t i) -> p t i", i=128),
                    axis=AX.X, op=ALU.min)
            bwdT.append(bw_s)

        # finals: sqrt + means
        for s in range(SPC):
            dfw = sb.tile([128, NIT], dt.float32, tag=f"df{s}")
            fsum = sb.tile([128, 1], dt.float32, tag=f"fs{s}")
            nc.scalar.activation(out=dfw[:, :], in_=fwdmin[s][:, :], func=AF.Sqrt,
                                 accum_out=fsum[:, :])
            dbw = sb.tile([128, NIT], dt.float32, tag=f"db{s}")
            bsum = sb.tile([128, 1], dt.float32, tag=f"bs{s}")
            nc.scalar.activation(out=dbw[:, :], in_=bwdT[s][:, :], func=AF.Sqrt,
                                 accum_out=bsum[:, :])
            tot = pp.tile([1, 1], dt.float32, tag="sc", bufs=1)
            nc.tensor.matmul(tot[:, :], lhsT=ones_t[:, :], rhs=fsum[:, :],
                             start=True, stop=False)
            nc.tensor.matmul(tot[:, :], lhsT=ones_t[:, :], rhs=bsum[:, :],
                             start=False, stop=True)
            sc_t = sb.tile([1, 1], dt.float32, tag=f"sc{s}")
            nc.scalar.mul(out=sc_t[:, :], in_=tot[:, :], mul=1.0 / (2 * NB))
            for q in range(4):
                nc.sync.dma_start(out=spat[s, :, q * 8:(q + 1) * 8],
                                  in_=dfw[:, q * 8:(q + 1) * 8])
            nc.sync.dma_start(out=scal[s], in_=sc_t[:, :])

    nc.compile()
    return nc


def _split3(x):
    """Split f32/f64 array into 3 bf16 pieces covering ~24 mantissa bits."""
    x = x.astype(np.float64)
    p1 = x.astype(BF16)
    r = x - p1.astype(np.float64)
    p2 = r.astype(BF16)
    r2 = r - p2.astype(np.float64)
    p3 = r2.astype(BF16)
    return p1, p2, p3


def _prep_sample(A, R):
    """A, R: (NB, 3) f32 -> amat (24, NB) bf16, bmat (24, NB) bf16.

    One K=24 bf16 matmul computes d2 = a2 + b2 - 2 a.b with all split
    cross terms down to ~2^-27:
      rows  0- 2: a2 pieces            x 1
      rows  3- 5: -2*a1_d              x b1_d
      rows  6- 8: 1                    x b2 pieces
      rows  9-11: -2*a1_d              x b2_d(piece2)
      rows 12-14: -2*a2(piece2)_d      x b1_d
      rows 15-17: -2*a1_d              x b3_d
      rows 18-20: -2*a2_d              x b2_d
      rows 21-23: -2*a3_d              x b1_d
    Row order keeps PSUM partials O(max(a2,b2)).
    """
    f64 = np.float64
    a1, a2_, a3 = _split3(A)
    b1, b2_, b3 = _split3(R)
    m2a1 = (a1.astype(f64) * -2.0).astype(BF16)  # exact
    m2a2 = (a2_.astype(f64) * -2.0).astype(BF16)
    m2a3 = (a3.astype(f64) * -2.0).astype(BF16)
    asq = (A.astype(f64) ** 2).sum(1)
    bsq = (R.astype(f64) ** 2).sum(1)
    aq1, aq2, aq3 = _split3(asq)
    bq1, bq2, bq3 = _split3(bsq)
    ones = np.ones(NB, dtype=BF16)

    arows = [aq1, aq2, aq3,
             m2a1[:, 0], m2a1[:, 1], m2a1[:, 2],
             ones, ones, ones,
             m2a1[:, 0], m2a1[:, 1], m2a1[:, 2],
             m2a2[:, 0], m2a2[:, 1], m2a2[:, 2],
             m2a1[:, 0], m2a1[:, 1], m2a1[:, 2],
             m2a2[:, 0], m2a2[:, 1], m2a2[:, 2],
             m2a3[:, 0], m2a3[:, 1], m2a3[:, 2]]
    brows = [ones, ones, ones,
             b1[:, 0], b1[:, 1], b1[:, 2],
             bq1, bq2, bq3,
             b2_[:, 0], b2_[:, 1], b2_[:, 2],
             b1[:, 0], b1[:, 1], b1[:, 2],
             b3[:, 0], b3[:, 1], b3[:, 2],
             b2_[:, 0], b2_[:, 1], b2_[:, 2],
             b1[:, 0], b1[:, 1], b1[:, 2]]
    amat = np.stack(arows)
    bmat = np.stack(brows)
    return np.ascontiguousarray(amat), np.ascontiguousarray(bmat)


def kernel(pos, reconstructed_pos, batch=None, **_unused):
    from concourse.bass_utils import run_bass_kernel_spmd

    pos = np.asarray(pos, dtype=np.float32).reshape(B, NB, D)
    rec = np.asarray(reconstructed_pos, dtype=np.float32).reshape(B, NB, D)

    if "nc" not in _CACHE:
        _CACHE["nc"] = _build_nc()
    nc = _CACHE["nc"]

    ident = np.eye(128, dtype=BF16)
    in_maps = []
    for c in range(NCORES):
        amats = np.empty((SPC, 24, NB), dtype=BF16)
        bmats = np.empty((SPC, 24, NB), dtype=BF16)
        for s in range(SPC):
            g = c * SPC + s
            amats[s], bmats[s] = _prep_sample(pos[g], rec[g])
        in_maps.append({"amat": amats, "bmat": bmats, "ident": ident})

    trace = bool(os.environ.get("BASS_TRACE"))
    res = run_bass_kernel_spmd(nc, in_maps, core_ids=list(range(NCORES)),
                               trace=trace)
    kernel.last_result = res

    coherence_scalar = np.empty((B, 1), dtype=np.float32)
    coherence_spatial = np.empty((B * NB,), dtype=np.float32)
    for c in range(NCORES):
        r = res.results[c]
        for s in range(SPC):
            g = c * SPC + s
            coherence_scalar[g, 0] = np.asarray(r["scal"])[s, 0]
            nat = np.asarray(r["spat"])[s]  # (128, 32): [p, it] -> i = it*128+p
            coherence_spatial[g * NB:(g + 1) * NB] = nat.T.reshape(NB)
    return coherence_scalar, coherence_spatial


# revision 28
# speedup vs baseline: 1.1536x; 1.1536x over previous
"""Per-sample Chamfer distance (B=16, NB=4096, 3D) on 8 TRN2 NeuronCores.

Data-parallel: 2 samples per core.  Per sample, the 4096x4096 squared
pairwise-distance matrix is produced on the TensorEngine as two bf16
matmuls per (128 x 512) tile (hi/lo-split operands, exact bf16 products
accumulated in f32 PSUM):

    d2[i,j] = a2[i] + b2[j] - 2*a[i].b[j]
            ~ MM1(K=7): -2ah.bh + 1*b2h + 1*b2l + a2h*1 + a2l*1
            + MM2(K=6): -2ah.bl + -2al.bh

ScalarE evacuates each PSUM tile to SBUF bf16 with Relu (the reference's
max(d2, 0) clamp).  VectorE then computes, per i-tile, the forward
row-min via a bf16 tensor_tensor min fold-chain (2x perf mode) and the
backward column-min by elementwise min-accumulation.  The backward
accumulator is finished with PE transposes + free-dim reduces, and both
directions get sqrt + mean on ScalarE.  min/sqrt commute, so reducing in
the d2 domain is exact up to bf16 rounding of the min itself.
"""

import os

import numpy as np
import ml_dtypes

B = 16
NB = 4096
D = 3
NCORES = 8
SPC = B // NCORES  # samples per core

BF16 = ml_dtypes.bfloat16

_CACHE = {}


def _build_nc():
    import concourse.bass as bass
    import concourse.bacc as bacc
    import concourse.mybir as mybir
    from concourse.tile import TileContext
    from contextlib import ExitStack

    dt = mybir.dt
    AF = mybir.ActivationFunctionType
    ALU = mybir.AluOpType
    AX = mybir.AxisListType

    NIT = NB // 128  # 32 i-tiles
    NJB = NB // 512  # 8 j-blocks

    nc = bacc.Bacc()

    amat = nc.declare_dram_parameter("amat", [SPC, 24, NB], dt.bfloat16, isOutput=False)
    bmat = nc.declare_dram_parameter("bmat", [SPC, 24, NB], dt.bfloat16, isOutput=False)
    ident = nc.declare_dram_parameter("ident", [128, 128], dt.bfloat16, isOutput=False)
    spat = nc.declare_dram_parameter("spat", [SPC, 128, NIT], dt.float32, isOutput=True)
    scal = nc.declare_dram_parameter("scal", [SPC, 1], dt.float32, isOutput=True)

    with TileContext(nc) as tc, ExitStack() as ctx:
        sb = ctx.enter_context(tc.tile_pool(name="sb", bufs=1))
        evp = ctx.enter_context(tc.tile_pool(name="ev", bufs=3))
        fp = ctx.enter_context(tc.tile_pool(name="fold", bufs=2))
        pp = ctx.enter_context(tc.tile_pool(name="ps", bufs=4, space="PSUM"))

        idt = sb.tile([128, 128], dt.bfloat16, tag="idt")
        nc.sync.dma_start(out=idt[:, :], in_=ident[:, :])
        ones_t = sb.tile([128, 1], dt.float32, tag="ones")
        nc.vector.memset(ones_t[:], 1.0)

        # PE warm-up primer: ~5us of back-to-back matmuls with no DMA deps
        # flips the HAM clock gate to 8/8 before the real work arrives; the
        # steady-state PE gaps are far below the re-throttle window, so the
        # array stays at 2.4 GHz for the whole kernel.
        wm = sb.tile([24, 512], dt.bfloat16, tag="wm")
        nc.vector.memset(wm[:, :], 0.0)
        wps = pp.tile([128, 512], dt.float32, tag="warm", bufs=1)
        for _ in range(24):
            nc.tensor.matmul(wps[:, :], lhsT=wm[:, 0:128], rhs=wm[:, :],
                             start=True, stop=True, tile_position=(0, 0))

        ta = []
        tb = []
        fwdmin = []
        acc = []
        for s in range(SPC):
            ta_s = sb.tile([64, NB], dt.bfloat16, tag=f"ta{s}")
            tb_s = sb.tile([64, NB], dt.bfloat16, tag=f"tb{s}")
            # replicate the 24 host rows at base partitions 0 and 32 (so
            # consecutive j-blocks run in two distinct PE row-groups,
            # concurrently).  Column-chunked DMAs: any 128-column weight
            # slice is covered by ONE dma (LDWEIGHTS caps sync-waits <4).
            for dram, tile in ((amat, ta_s), (bmat, tb_s)):
                for c in range(0, NB, 1024):
                    for o in (0, 32):
                        nc.sync.dma_start(out=tile[o:o + 24, c:c + 1024],
                                          in_=dram[s, :, c:c + 1024])
            ta.append(ta_s)
            tb.append(tb_s)
            fwdmin.append(sb.tile([128, NIT], dt.float32, tag=f"fw{s}", name=f"fw{s}"))
            acc.append(sb.tile([128, NB], dt.bfloat16, tag=f"acc{s}", name=f"acc{s}"))

        for s in range(SPC):
            ta_s, tb_s, fw_s, acc_s = ta[s], tb[s], fwdmin[s], acc[s]
            for it in range(NIT):
                isl = slice(it * 128, (it + 1) * 128)
                ev = evp.tile([128, NB], dt.bfloat16, tag="ev", bufs=4)
                for jb in range(NJB):
                    jsl = slice(jb * 512, (jb + 1) * 512)
                    # alternate row-groups 0/1 between j-blocks (concurrent MMs)
                    o = 0 if jb % 2 == 0 else 32
                    ps = pp.tile([128, 512], dt.float32, tag="d2", bufs=5)
                    nc.tensor.matmul(ps[:, :], lhsT=ta_s[o:o + 24, isl],
                                     rhs=tb_s[o:o + 24, jsl],
                                     start=True, stop=True,
                                     tile_position=(o, 0))
                    nc.scalar.activation(out=ev[:, jsl], in_=ps[:, :], func=AF.Relu)
                # backward: elementwise min-accumulate across i-tiles
                if it == 0:
                    nc.gpsimd.tensor_copy(acc_s[:, :], ev[:, :])
                else:
                    nc.vector.tensor_tensor(out=acc_s[:, :], in0=ev[:, :],
                                            in1=acc_s[:, :], op=ALU.min)
                # forward: bf16 fold chain (2x mode) down to 256, then reduce
                f1 = fp.tile([128, 2048], dt.bfloat16, tag="f1")
                nc.vector.tensor_tensor(out=f1[:, :], in0=ev[:, 0:2048],
                                        in1=ev[:, 2048:4096], op=ALU.min)
                f2 = fp.tile([128, 1024], dt.bfloat16, tag="f2")
                nc.vector.tensor_tensor(out=f2[:, :], in0=f1[:, 0:1024],
                                        in1=f1[:, 1024:2048], op=ALU.min)
                f3 = fp.tile([128, 512], dt.bfloat16, tag="f3")
                nc.vector.tensor_tensor(out=f3[:, :], in0=f2[:, 0:512],
                                        in1=f2[:, 512:1024], op=ALU.min)
                f4 = fp.tile([128, 256], dt.bfloat16, tag="f4")
                nc.vector.tensor_tensor(out=f4[:, :], in0=f3[:, 0:256],
                                        in1=f3[:, 256:512], op=ALU.min)
                nc.vector.tensor_reduce(out=fw_s[:, it:it + 1], in_=f4[:, :],
                                        axis=AX.X, op=ALU.min)

            # backward finish for this sample (overlaps next sample's main
            # loop): PE-transpose the accumulator, reduce along free dim
            bw_s = sb.tile([128, NIT], dt.float32, tag=f"bw{s}", name=f"bw{s}")
            for g in range(4):
                tp = pp.tile([128, 1024], dt.bfloat16, tag="tp", bufs=1)
                for t8 in range(8):
                    t = g * 8 + t8
                    nc.tensor.transpose(tp[:, t8 * 128:(t8 + 1) * 128],
                                        acc_s[:, t * 128:(t + 1) * 128], idt[:, :])
                nc.vector.tensor_reduce(
                    out=bw_s[:, g * 8:(g + 1) * 8],
                    in_=tp[:, :].rearrange("p (t i) -> p t i", i=128),
                    axis=AX.X, op=ALU.min)

            # finals: sqrt + means
            dfw = sb.tile([128, NIT], dt.float32, tag=f"df{s}", name=f"df{s}")
            fsum = sb.tile([128, 1], dt.float32, tag=f"fs{s}", name=f"fs{s}")
            nc.scalar.activation(out=dfw[:, :], in_=fw_s[:, :], func=AF.Sqrt,
                                 accum_out=fsum[:, :])
            dbw = sb.tile([128, NIT], dt.float32, tag=f"db{s}", name=f"db{s}")
            bsum = sb.tile([128, 1], dt.float32, tag=f"bs{s}", name=f"bs{s}")
            nc.scalar.activation(out=dbw[:, :], in_=bw_s[:, :], func=AF.Sqrt,
                                 accum_out=bsum[:, :])
            tot = pp.tile([1, 1], dt.float32, tag="sc", bufs=1)
            nc.tensor.matmul(tot[:, :], lhsT=ones_t[:, :], rhs=fsum[:, :],
                             start=True, stop=False)
            nc.tensor.matmul(tot[:, :], lhsT=ones_t[:, :], rhs=bsum[:, :],
                             start=False, stop=True)
            sc_t = sb.tile([1, 1], dt.float32, tag=f"sc{s}", name=f"sc{s}")
            nc.scalar.mul(out=sc_t[:, :], in_=tot[:, :], mul=1.0 / (2 * NB))
            for q in range(4):
                nc.sync.dma_start(out=spat[s, :, q * 8:(q + 1) * 8],
                                  in_=dfw[:, q * 8:(q + 1) * 8])
            nc.sync.dma_start(out=scal[s], in_=sc_t[:, :])

    nc.compile()
    return nc


def _split3(x):
    """Split f32/f64 array into 3 bf16 pieces covering ~24 mantissa bits."""
    x = x.astype(np.float64)
    p1 = x.astype(BF16)
    r = x - p1.astype(np.float64)
    p2 = r.astype(BF16)
    r2 = r - p2.astype(np.float64)
    p3 = r2.astype(BF16)
    return p1, p2, p3


def _prep_sample(A, R):
    """A, R: (NB, 3) f32 -> amat (24, NB) bf16, bmat (24, NB) bf16.

    One K=24 bf16 matmul computes d2 = a2 + b2 - 2 a.b with all split
    cross terms down to ~2^-27:
      rows  0- 2: a2 pieces            x 1
      rows  3- 5: -2*a1_d              x b1_d
      rows  6- 8: 1                    x b2 pieces
      rows  9-11: -2*a1_d              x b2_d(piece2)
      rows 12-14: -2*a2(piece2)_d      x b1_d
      rows 15-17: -2*a1_d              x b3_d
      rows 18-20: -2*a2_d              x b2_d
      rows 21-23: -2*a3_d              x b1_d
    Row order keeps PSUM partials O(max(a2,b2)).
    """
    f64 = np.float64
    a1, a2_, a3 = _split3(A)
    b1, b2_, b3 = _split3(R)
    m2a1 = (a1.astype(f64) * -2.0).astype(BF16)  # exact
    m2a2 = (a2_.astype(f64) * -2.0).astype(BF16)
    m2a3 = (a3.astype(f64) * -2.0).astype(BF16)
    asq = (A.astype(f64) ** 2).sum(1)
    bsq = (R.astype(f64) ** 2).sum(1)
    aq1, aq2, aq3 = _split3(asq)
    bq1, bq2, bq3 = _split3(bsq)
    ones = np.ones(NB, dtype=BF16)

    arows = [aq1, aq2, aq3,
             m2a1[:, 0], m2a1[:, 1], m2a1[:, 2],
             ones, ones, ones,
             m2a1[:, 0], m2a1[:, 1], m2a1[:, 2],
             m2a2[:, 0], m2a2[:, 1], m2a2[:, 2],
             m2a1[:, 0], m2a1[:, 1], m2a1[:, 2],
             m2a2[:, 0], m2a2[:, 1], m2a2[:, 2],
             m2a3[:, 0], m2a3[:, 1], m2a3[:, 2]]
    brows = [ones, ones, ones,
             b1[:, 0], b1[:, 1], b1[:, 2],
             bq1, bq2, bq3,
             b2_[:, 0], b2_[:, 1], b2_[:, 2],
             b1[:, 0], b1[:, 1], b1[:, 2],
             b3[:, 0], b3[:, 1], b3[:, 2],
             b2_[:, 0], b2_[:, 1], b2_[:, 2],
             b1[:, 0], b1[:, 1], b1[:, 2]]
    amat = np.stack(arows)
    bmat = np.stack(brows)
    return np.ascontiguousarray(amat), np.ascontiguousarray(bmat)


def kernel(pos, reconstructed_pos, batch=None, **_unused):
    from concourse.bass_utils import run_bass_kernel_spmd

    pos = np.asarray(pos, dtype=np.float32).reshape(B, NB, D)
    rec = np.asarray(reconstructed_pos, dtype=np.float32).reshape(B, NB, D)

    if "nc" not in _CACHE:
        _CACHE["nc"] = _build_nc()
    nc = _CACHE["nc"]

    ident = np.eye(128, dtype=BF16)
    in_maps = []
    for c in range(NCORES):
        amats = np.empty((SPC, 24, NB), dtype=BF16)
        bmats = np.empty((SPC, 24, NB), dtype=BF16)
        for s in range(SPC):
            g = c * SPC + s
            amats[s], bmats[s] = _prep_sample(pos[g], rec[g])
        in_maps.append({"amat": amats, "bmat": bmats, "ident": ident})

    trace = bool(os.environ.get("BASS_TRACE"))
    res = run_bass_kernel_spmd(nc, in_maps, core_ids=list(range(NCORES)),
                               trace=trace)
    kernel.last_result = res

    coherence_scalar = np.empty((B, 1), dtype=np.float32)
    coherence_spatial = np.empty((B * NB,), dtype=np.float32)
    for c in range(NCORES):
        r = res.results[c]
        for s in range(SPC):
            g = c * SPC + s
            coherence_scalar[g, 0] = np.asarray(r["scal"])[s, 0]
            nat = np.asarray(r["spat"])[s]  # (128, 32): [p, it] -> i = it*128+p
            coherence_spatial[g * NB:(g + 1) * NB] = nat.T.reshape(NB)
    return coherence_scalar, coherence_spatial


# revision 29
# speedup vs baseline: 1.2337x; 1.0694x over previous
"""Per-sample Chamfer distance (B=16, NB=4096, 3D) on 8 TRN2 NeuronCores.

Data-parallel: 2 samples per core.  Per sample, the 4096x4096 squared
pairwise-distance matrix is produced on the TensorEngine as two bf16
matmuls per (128 x 512) tile (hi/lo-split operands, exact bf16 products
accumulated in f32 PSUM):

    d2[i,j] = a2[i] + b2[j] - 2*a[i].b[j]
            ~ MM1(K=7): -2ah.bh + 1*b2h + 1*b2l + a2h*1 + a2l*1
            + MM2(K=6): -2ah.bl + -2al.bh

ScalarE evacuates each PSUM tile to SBUF bf16 with Relu (the reference's
max(d2, 0) clamp).  VectorE then computes, per i-tile, the forward
row-min via a bf16 tensor_tensor min fold-chain (2x perf mode) and the
backward column-min by elementwise min-accumulation.  The backward
accumulator is finished with PE transposes + free-dim reduces, and both
directions get sqrt + mean on ScalarE.  min/sqrt commute, so reducing in
the d2 domain is exact up to bf16 rounding of the min itself.
"""

import os

import numpy as np
import ml_dtypes

B = 16
NB = 4096
D = 3
NCORES = 8
SPC = B // NCORES  # samples per core

BF16 = ml_dtypes.bfloat16

_CACHE = {}


def _build_nc():
    import concourse.bass as bass
    import concourse.bacc as bacc
    import concourse.mybir as mybir
    from concourse.tile import TileContext
    from contextlib import ExitStack

    dt = mybir.dt
    AF = mybir.ActivationFunctionType
    ALU = mybir.AluOpType
    AX = mybir.AxisListType

    NIT = NB // 128  # 32 i-tiles
    NJB = NB // 512  # 8 j-blocks

    nc = bacc.Bacc()

    amat = nc.declare_dram_parameter("amat", [SPC, 24, NB], dt.bfloat16, isOutput=False)
    bmat = nc.declare_dram_parameter("bmat", [SPC, 24, NB], dt.bfloat16, isOutput=False)
    ident = nc.declare_dram_parameter("ident", [128, 128], dt.bfloat16, isOutput=False)
    spat = nc.declare_dram_parameter("spat", [SPC, 128, NIT], dt.float32, isOutput=True)
    scal = nc.declare_dram_parameter("scal", [SPC, 1], dt.float32, isOutput=True)

    with TileContext(nc) as tc, ExitStack() as ctx:
        sb = ctx.enter_context(tc.tile_pool(name="sb", bufs=1))
        evp = ctx.enter_context(tc.tile_pool(name="ev", bufs=3))
        fp = ctx.enter_context(tc.tile_pool(name="fold", bufs=2))
        pp = ctx.enter_context(tc.tile_pool(name="ps", bufs=4, space="PSUM"))

        idt = sb.tile([128, 128], dt.bfloat16, tag="idt")
        nc.sync.dma_start(out=idt[:, :], in_=ident[:, :])
        ones_t = sb.tile([128, 1], dt.float32, tag="ones")
        nc.vector.memset(ones_t[:], 1.0)

        # PE warm-up primer: ~5us of back-to-back matmuls with no DMA deps
        # flips the HAM clock gate to 8/8 before the real work arrives; the
        # steady-state PE gaps are far below the re-throttle window, so the
        # array stays at 2.4 GHz for the whole kernel.
        wm = sb.tile([24, 512], dt.bfloat16, tag="wm")
        nc.vector.memset(wm[:, :], 0.0)
        wps = pp.tile([128, 512], dt.float32, tag="warm", bufs=1)
        for _ in range(24):
            nc.tensor.matmul(wps[:, :], lhsT=wm[:, 0:128], rhs=wm[:, :],
                             start=True, stop=True, tile_position=(0, 0))

        ta = []
        tb = []
        fwdmin = []
        acc = []
        for s in range(SPC):
            ta_s = sb.tile([64, NB], dt.bfloat16, tag=f"ta{s}")
            tb_s = sb.tile([64, NB], dt.bfloat16, tag=f"tb{s}")
            # replicate the 24 host rows at base partitions 0 and 32 (so
            # consecutive j-blocks run in two distinct PE row-groups,
            # concurrently).  Column-chunked DMAs: any 128-column weight
            # slice is covered by ONE dma (LDWEIGHTS caps sync-waits <4).
            for dram, tile in ((amat, ta_s), (bmat, tb_s)):
                for c in range(0, NB, 1024):
                    for o in (0, 32):
                        nc.sync.dma_start(out=tile[o:o + 24, c:c + 1024],
                                          in_=dram[s, :, c:c + 1024])
            ta.append(ta_s)
            tb.append(tb_s)
            fwdmin.append(sb.tile([128, NIT], dt.float32, tag=f"fw{s}", name=f"fw{s}"))
            acc.append(sb.tile([128, NB], dt.bfloat16, tag=f"acc{s}", name=f"acc{s}"))

        for s in range(SPC):
            ta_s, tb_s, fw_s, acc_s = ta[s], tb[s], fwdmin[s], acc[s]
            for it in range(NIT):
                isl = slice(it * 128, (it + 1) * 128)
                ev = evp.tile([128, NB], dt.bfloat16, tag="ev", bufs=4)
                for jb in range(NJB):
                    jsl = slice(jb * 512, (jb + 1) * 512)
                    # alternate row-groups 0/1 between j-blocks (concurrent MMs)
                    o = 0 if jb % 2 == 0 else 32
                    ps = pp.tile([128, 512], dt.float32, tag="d2", bufs=5)
                    nc.tensor.matmul(ps[:, :], lhsT=ta_s[o:o + 24, isl],
                                     rhs=tb_s[o:o + 24, jsl],
                                     start=True, stop=True,
                                     tile_position=(o, 0))
                    nc.scalar.activation(out=ev[:, jsl], in_=ps[:, :], func=AF.Relu)
                # backward: elementwise min-accumulate across i-tiles
                if it == 0:
                    nc.vector.tensor_copy(acc_s[:, :], ev[:, :])
                else:
                    nc.vector.tensor_tensor(out=acc_s[:, :], in0=ev[:, :],
                                            in1=acc_s[:, :], op=ALU.min)
                # forward: bf16 fold chain (2x mode) down to 256, then reduce
                f1 = fp.tile([128, 2048], dt.bfloat16, tag="f1")
                nc.vector.tensor_tensor(out=f1[:, :], in0=ev[:, 0:2048],
                                        in1=ev[:, 2048:4096], op=ALU.min)
                f2 = fp.tile([128, 1024], dt.bfloat16, tag="f2")
                nc.vector.tensor_tensor(out=f2[:, :], in0=f1[:, 0:1024],
                                        in1=f1[:, 1024:2048], op=ALU.min)
                f3 = fp.tile([128, 512], dt.bfloat16, tag="f3")
                nc.vector.tensor_tensor(out=f3[:, :], in0=f2[:, 0:512],
                                        in1=f2[:, 512:1024], op=ALU.min)
                f4 = fp.tile([128, 256], dt.bfloat16, tag="f4")
                nc.vector.tensor_tensor(out=f4[:, :], in0=f3[:, 0:256],
                                        in1=f3[:, 256:512], op=ALU.min)
                nc.vector.tensor_reduce(out=fw_s[:, it:it + 1], in_=f4[:, :],
                                        axis=AX.X, op=ALU.min)

            # backward finish for this sample (overlaps next sample's main
            # loop): PE-transpose the accumulator, reduce along free dim
            bw_s = sb.tile([128, NIT], dt.float32, tag=f"bw{s}", name=f"bw{s}")
            for g in range(4):
                tp = pp.tile([128, 1024], dt.bfloat16, tag="tp", bufs=1)
                for t8 in range(8):
                    t = g * 8 + t8
                    nc.tensor.transpose(tp[:, t8 * 128:(t8 + 1) * 128],
                                        acc_s[:, t * 128:(t + 1) * 128], idt[:, :])
                nc.vector.tensor_reduce(
                    out=bw_s[:, g * 8:(g + 1) * 8],
                    in_=tp[:, :].rearrange("p (t i) -> p t i", i=128),
                    axis=AX.X, op=ALU.min)

            # finals: sqrt + means
            dfw = sb.tile([128, NIT], dt.float32, tag=f"df{s}", name=f"df{s}")
            fsum = sb.tile([128, 1], dt.float32, tag=f"fs{s}", name=f"fs{s}")
            nc.scalar.activation(out=dfw[:, :], in_=fw_s[:, :], func=AF.Sqrt,
                                 accum_out=fsum[:, :])
            dbw = sb.tile([128, NIT], dt.float32, tag=f"db{s}", name=f"db{s}")
            bsum = sb.tile([128, 1], dt.float32, tag=f"bs{s}", name=f"bs{s}")
            nc.scalar.activation(out=dbw[:, :], in_=bw_s[:, :], func=AF.Sqrt,
                                 accum_out=bsum[:, :])
            tot = pp.tile([1, 1], dt.float32, tag="sc", bufs=1)
            nc.tensor.matmul(tot[:, :], lhsT=ones_t[:, :], rhs=fsum[:, :],
                             start=True, stop=False)
            nc.tensor.matmul(tot[:, :], lhsT=ones_t[:, :], rhs=bsum[:, :],
                             start=False, stop=True)
            sc_t = sb.tile([1, 1], dt.float32, tag=f"sc{s}", name=f"sc{s}")
            nc.scalar.mul(out=sc_t[:, :], in_=tot[:, :], mul=1.0 / (2 * NB))
            for q in range(4):
                nc.sync.dma_start(out=spat[s, :, q * 8:(q + 1) * 8],
                                  in_=dfw[:, q * 8:(q + 1) * 8])
            nc.sync.dma_start(out=scal[s], in_=sc_t[:, :])

    nc.compile()
    return nc


def _split3(x):
    """Split f32/f64 array into 3 bf16 pieces covering ~24 mantissa bits."""
    x = x.astype(np.float64)
    p1 = x.astype(BF16)
    r = x - p1.astype(np.float64)
    p2 = r.astype(BF16)
    r2 = r - p2.astype(np.float64)
    p3 = r2.astype(BF16)
    return p1, p2, p3


def _prep_sample(A, R):
    """A, R: (NB, 3) f32 -> amat (24, NB) bf16, bmat (24, NB) bf16.

    One K=24 bf16 matmul computes d2 = a2 + b2 - 2 a.b with all split
    cross terms down to ~2^-27:
      rows  0- 2: a2 pieces            x 1
      rows  3- 5: -2*a1_d              x b1_d
      rows  6- 8: 1                    x b2 pieces
      rows  9-11: -2*a1_d              x b2_d(piece2)
      rows 12-14: -2*a2(piece2)_d      x b1_d
      rows 15-17: -2*a1_d              x b3_d
      rows 18-20: -2*a2_d              x b2_d
      rows 21-23: -2*a3_d              x b1_d
    Row order keeps PSUM partials O(max(a2,b2)).
    """
    f64 = np.float64
    a1, a2_, a3 = _split3(A)
    b1, b2_, b3 = _split3(R)
    m2a1 = (a1.astype(f64) * -2.0).astype(BF16)  # exact
    m2a2 = (a2_.astype(f64) * -2.0).astype(BF16)
    m2a3 = (a3.astype(f64) * -2.0).astype(BF16)
    asq = (A.astype(f64) ** 2).sum(1)
    bsq = (R.astype(f64) ** 2).sum(1)
    aq1, aq2, aq3 = _split3(asq)
    bq1, bq2, bq3 = _split3(bsq)
    ones = np.ones(NB, dtype=BF16)

    arows = [aq1, aq2, aq3,
             m2a1[:, 0], m2a1[:, 1], m2a1[:, 2],
             ones, ones, ones,
             m2a1[:, 0], m2a1[:, 1], m2a1[:, 2],
             m2a2[:, 0], m2a2[:, 1], m2a2[:, 2],
             m2a1[:, 0], m2a1[:, 1], m2a1[:, 2],
             m2a2[:, 0], m2a2[:, 1], m2a2[:, 2],
             m2a3[:, 0], m2a3[:, 1], m2a3[:, 2]]
    brows = [ones, ones, ones,
             b1[:, 0], b1[:, 1], b1[:, 2],
             bq1, bq2, bq3,
             b2_[:, 0], b2_[:, 1], b2_[:, 2],
             b1[:, 0], b1[:, 1], b1[:, 2],
             b3[:, 0], b3[:, 1], b3[:, 2],
             b2_[:, 0], b2_[:, 1], b2_[:, 2],
             b1[:, 0], b1[:, 1], b1[:, 2]]
    amat = np.stack(arows)
    bmat = np.stack(brows)
    return np.ascontiguousarray(amat), np.ascontiguousarray(bmat)


def kernel(pos, reconstructed_pos, batch=None, **_unused):
    from concourse.bass_utils import run_bass_kernel_spmd

    pos = np.asarray(pos, dtype=np.float32).reshape(B, NB, D)
    rec = np.asarray(reconstructed_pos, dtype=np.float32).reshape(B, NB, D)

    if "nc" not in _CACHE:
        _CACHE["nc"] = _build_nc()
    nc = _CACHE["nc"]

    ident = np.eye(128, dtype=BF16)
    in_maps = []
    for c in range(NCORES):
        amats = np.empty((SPC, 24, NB), dtype=BF16)
        bmats = np.empty((SPC, 24, NB), dtype=BF16)
        for s in range(SPC):
            g = c * SPC + s
            amats[s], bmats[s] = _prep_sample(pos[g], rec[g])
        in_maps.append({"amat": amats, "bmat": bmats, "ident": ident})

    trace = bool(os.environ.get("BASS_TRACE"))
    res = run_bass_kernel_spmd(nc, in_maps, core_ids=list(range(NCORES)),
                               trace=trace)
    kernel.last_result = res

    coherence_scalar = np.empty((B, 1), dtype=np.float32)
    coherence_spatial = np.empty((B * NB,), dtype=np.float32)
    for c in range(NCORES):
        r = res.results[c]
        for s in range(SPC):
            g = c * SPC + s
            coherence_scalar[g, 0] = np.asarray(r["scal"])[s, 0]
            nat = np.asarray(r["spat"])[s]  # (128, 32): [p, it] -> i = it*128+p
            coherence_spatial[g * NB:(g + 1) * NB] = nat.T.reshape(NB)
    return coherence_scalar, coherence_spatial
